# revision 41
# baseline (speedup 1.0000x reference)
"""Trainium2 Bass kernel for GQA attention forward (B=2, S=2048, D=2048,
16 q-heads / 4 kv-heads, head_dim=128, RoPE, causal).

Sharding: 8 cores = 2 (batch) x 4 (kv-head groups). Each core computes its
batch's attention for one kv-head group (4 q-heads + 1 kv head) and a
row-parallel partial of the output projection; the host sums the 4 partials
per batch.

v5 structure -- one fully-merged software pipeline:
- Weight-stationary projections (stationary = weight d-tile, moving = x
  s-chunk) produce Q/K directly in [e, s] layout; the RoPE half-swap runs
  as a PE permutation matmul (DVE lanes cannot cross partitions).
- Attention is emitted as per-(chunk, head) generators advanced a few
  k-tile steps after every projection column, so the exp-bound attention
  stream (Scalar ~600ns/tile vs PE ~360ns/tile) overlaps the PE-bound
  projection stream from the second s-chunk onward.
- Output-projection pieces (4 matmuls + PSUM->SBUF bf16 copy) drain through
  the same step hooks; copies alternate Scalar/Vector engines.
- Softmax denominator: 4-way col-tiled M=1 matmuls + one f32r `sel` matmul
  (rows {0,32,64,96} ones) that reduces partials and broadcasts in one shot.
- PSUM: proj+swap+v-transpose share a 2-ring; scores+denominator-broadcast
  share a 2-ring; ps_o is a single buffer with normalization inline at unit
  end; out-proj pieces use a 2-ring; denominator accumulator 1 bank = 8.
- Output partials are written bf16 (host accumulates in fp32).
"""

import sys

if "/opt/trn_rl_repo" not in sys.path:
    sys.path.insert(0, "/opt/trn_rl_repo")

import numpy as np
import ml_dtypes

import concourse.bass as bass
import concourse.tile as tile
from concourse import mybir

F32 = mybir.dt.float32
F32R = mybir.dt.float32r
BF16 = mybir.dt.bfloat16

# Full-problem constants (per reference).
B, S, DIM = 2, 2048, 2048
N_HEADS, N_KV_HEADS, HEAD_DIM = 16, 4, 128
N_GROUPS = N_KV_HEADS          # tensor-parallel groups
HQ = N_HEADS // N_KV_HEADS     # q heads per group
NEG = -1e30


def build_attention_core(nc, S=S, D=DIM, HQ=HQ, HD=HEAD_DIM, CHUNK=512):
    n_st = S // 128        # s tiles
    n_dt = D // 128        # d tiles
    n_ch = S // CHUNK      # q chunks
    kpc = CHUNK // 128     # k-tiles per chunk
    n_dc = D // CHUNK      # d chunks (out-proj)
    spc = CHUNK // 128     # s-tiles per chunk
    IQ = HQ * HD

    x_d = nc.dram_tensor("xT", [128, n_ch, n_dt, CHUNK], BF16, kind="ExternalInput")
    wqT_d = nc.dram_tensor("wqT", [128, HQ, n_dt, HD], BF16, kind="ExternalInput")
    wkvT_d = nc.dram_tensor("wkvT", [128, 2, n_dt, HD], BF16, kind="ExternalInput")
    woT_d = nc.dram_tensor("woT", [128, IQ // 128, D], BF16, kind="ExternalInput")
    t1T_d = nc.dram_tensor("t1T", [128, S], BF16, kind="ExternalInput")
    t2T_d = nc.dram_tensor("t2T", [128, S], F32, kind="ExternalInput")
    mask0_d = nc.dram_tensor("mask0", [128, CHUNK], F32, kind="ExternalInput")
    ident_d = nc.dram_tensor("ident", [128, 128], BF16, kind="ExternalInput")
    pswap_d = nc.dram_tensor("pswap", [128, 128], BF16, kind="ExternalInput")
    onesc_d = nc.dram_tensor("ones_col", [128, 1], BF16, kind="ExternalInput")
    onesr_d = nc.dram_tensor("ones_row", [1, 128], F32R, kind="ExternalInput")
    out_d = nc.dram_tensor("out_partial", [S, D], BF16, kind="ExternalOutput")

    scale = float(HD) ** -0.5

    with tile.TileContext(nc) as tc:
        with (
            tc.tile_pool(name="persist", bufs=1) as persist,
            tc.tile_pool(name="constB", bufs=1) as constB,
        ):
            qT_sb = persist.tile([128, HQ, S], BF16)    # [e, h, s]
            kT_sb = persist.tile([128, S], BF16)        # [e, s]
            v_sb = persist.tile([128, n_st, HD], BF16)  # [s_in_tile, s_tile, e]
            oT_sb = persist.tile([128, HQ, S], BF16)    # [e, h, s]

            # weights / tables on the scalar queue, split so the first
            # matmuls only wait on their own slice; x on the sync queue.
            wq_sb = persist.tile([128, HQ, n_dt, HD], BF16)
            for wp in range(4):
                stp = n_dt // 4
                nc.sync.dma_start(
                    out=wq_sb[:, 0, wp * stp:(wp + 1) * stp],
                    in_=wqT_d[:, 0, wp * stp:(wp + 1) * stp],
                )
            wkv_sb = persist.tile([128, 2, n_dt, HD], BF16)
            t1T_sb = persist.tile([128, S], BF16)
            nc.scalar.dma_start(out=t1T_sb, in_=t1T_d[:])
            t2T_sb = persist.tile([128, S], F32)
            nc.scalar.dma_start(out=t2T_sb, in_=t2T_d[:])
            ident = persist.tile([128, 128], BF16)
            nc.scalar.dma_start(out=ident, in_=ident_d[:])
            pswap_sb = persist.tile([128, 128], BF16)
            nc.scalar.dma_start(out=pswap_sb, in_=pswap_d[:])
            mask0_sb = constB.tile([128, CHUNK], F32)
            nc.scalar.dma_start(out=mask0_sb, in_=mask0_d[:])
            ones_col = constB.tile([128, 1], BF16)
            nc.scalar.dma_start(out=ones_col, in_=onesc_d[:])
            ones_row = constB.tile([1, 128], F32R)
            nc.scalar.dma_start(out=ones_row, in_=onesr_d[:])
            woT_sb = persist.tile([128, IQ // 128, D], BF16)
            nc.scalar.dma_start(out=woT_sb, in_=woT_d[:])

            with (
                tc.tile_pool(name="xslab", bufs=2) as xpool,
                tc.tile_pool(name="rope", bufs=2) as rope_pool,
                tc.tile_pool(name="expt", bufs=6) as expt_pool,
                tc.tile_pool(name="maskbuf", bufs=3) as mask_pool,
                tc.tile_pool(name="sums", bufs=2) as sums_pool,
                tc.tile_pool(name="recip", bufs=2) as rec_pool,
                tc.tile_pool(name="outsb", bufs=2) as outsb_pool,
                tc.tile_pool(name="scratch", bufs=1) as scratch_pool,
                # PSUM: exactly 8 banks
                tc.tile_pool(name="psP", bufs=2, space="PSUM") as psP,
                tc.tile_pool(name="psS", bufs=3, space="PSUM") as psS,
                tc.tile_pool(name="psO", bufs=1, space="PSUM") as psO,
                tc.tile_pool(name="psX", bufs=2, space="PSUM") as psX,
                tc.tile_pool(name="accp", bufs=2) as acc_pool,
            ):
                # warm the exp spline tables during the DMA fill
                warm = scratch_pool.tile([1, 1], BF16)
                nc.scalar.activation(
                    warm, mask0_sb[0:1, 0:1],
                    mybir.ActivationFunctionType.Exp,
                )

                cq = []           # out-proj pieces ready: (st, dc)
                out_tiles = {}    # st -> out_sb tile
                seeded = [0]

                def emit_c_piece():
                    st, dc = cq.pop(0)
                    if dc == 0:
                        out_tiles[st] = outsb_pool.tile(
                            [128, D], BF16, tag="outsb", name=f"out_sb{st}"
                        )
                    out_sb = out_tiles[st]
                    st_sl = slice(st * 128, (st + 1) * 128)
                    ps_d = psX.tile([128, CHUNK], F32, tag="psd")
                    for it in range(HQ):
                        nc.tensor.matmul(
                            ps_d,
                            oT_sb[:, it, st_sl],
                            woT_sb[:, it, dc * CHUNK:(dc + 1) * CHUNK],
                            start=(it == 0), stop=(it == HQ - 1),
                        )
                    if dc % 2 == 0:
                        nc.vector.tensor_copy(
                            out_sb[:, dc * CHUNK:(dc + 1) * CHUNK], ps_d
                        )
                    else:
                        nc.scalar.copy(
                            out_sb[:, dc * CHUNK:(dc + 1) * CHUNK], ps_d
                        )
                    if dc == n_dc - 1:
                        nc.gpsimd.dma_start(out=out_d[st_sl, :], in_=out_sb)
                        del out_tiles[st]

                st_norm = [None]

                def emit_norm():
                    ps_sum1, ps_o_, h_, c_ = st_norm[0]
                    st_norm[0] = None
                    sums_sb = sums_pool.tile([1, CHUNK], F32R, tag="sums")
                    with nc.allow_low_precision(reason="f32r denom"):
                        nc.vector.tensor_copy(sums_sb, ps_sum1)
                    ps_b = psS.tile(
                        [128, CHUNK], F32, tag="ps_s", name=f"ps_b_{c_}_{h_}"
                    )
                    nc.tensor.matmul(ps_b, ones_row, sums_sb, start=True, stop=True)
                    recip = rec_pool.tile([128, CHUNK], F32, tag="recip")
                    nc.vector.reciprocal_approx_fast(recip, ps_b)
                    nc.vector.tensor_mul(
                        oT_sb[:, h_, c_ * CHUNK:(c_ + 1) * CHUNK], ps_o_, recip,
                    )
                    if h_ == HQ - 1:
                        for sti in range(spc):
                            for dc in range(n_dc):
                                cq.append((c_ * spc + sti, dc))

                def attn_unit_gen(c, h):
                    """Generator: one yield per k-tile step."""
                    ps_o = psO.tile(
                        [128, CHUNK], F32, tag="ps_o", name=f"ps_o_{c}_{h}"
                    )
                    # denominator accumulator on the Vector engine (bf16 2x)
                    acc = acc_pool.tile(
                        [128, CHUNK], BF16, tag="acc", name=f"acc_{c}_{h}"
                    )
                    n_kj = (c + 1) * kpc
                    pending = []

                    def flush_av():
                        pe, pj, poff = pending.pop(0)
                        nc.tensor.matmul(
                            ps_o[:, poff:], v_sb[:, pj, :], pe,
                            start=(pj == 0), stop=(pj == n_kj - 1),
                        )

                    for kj in range(n_kj):
                        # ready work first: AV of kj-4 (exp long done) and
                        # an out-proj piece execute while the next score
                        # matmul would stall on its PSUM ring slot
                        if len(pending) > 3:
                            flush_av()
                        if cq:
                            emit_c_piece()
                        off = max(0, (kj - c * kpc)) * 128
                        w = CHUNK - off
                        ps_s = psS.tile(
                            [128, CHUNK], F32, tag="ps_s", name=f"ps_s{c}{h}{kj}"
                        )
                        nc.tensor.matmul(
                            ps_s[:, 0:w],
                            kT_sb[:, kj * 128:(kj + 1) * 128],
                            qT_sb[:, h, c * CHUNK + off:(c + 1) * CHUNK],
                            start=True, stop=True,
                        )
                        if kj >= c * kpc:  # diagonal block: causal mask
                            msk = mask_pool.tile([128, CHUNK], F32, tag="msk")
                            nc.vector.tensor_add(
                                msk[:, 0:w], ps_s[:, 0:w], mask0_sb[:, 0:w],
                            )
                            exp_in = msk
                        else:
                            exp_in = ps_s
                        expT = expt_pool.tile([128, CHUNK], BF16, tag="expT")
                        nc.scalar.activation(
                            expT[:, 0:w], exp_in[:, 0:w],
                            mybir.ActivationFunctionType.Exp,
                            scale=scale,
                        )
                        if kj == 0:
                            nc.vector.tensor_copy(acc, expT)
                        else:
                            nc.vector.tensor_add(
                                acc[:, off:], acc[:, off:], expT[:, 0:w]
                            )
                        pending.append((expT[:, 0:w], kj, off))
                        if kj == 1 and st_norm[0] is not None:
                            emit_norm()
                        yield
                    while pending:
                        flush_av()

                    # k-partition reduce of the accumulated exp: one M=1 matmul
                    ps_sum1 = psS.tile(
                        [1, CHUNK], F32, tag="ps_s", name=f"ps_sum1_{c}_{h}"
                    )
                    nc.tensor.matmul(ps_sum1, ones_col, acc, start=True, stop=True)
                    st_norm[0] = (ps_sum1, ps_o, h, c)

                from collections import deque

                agen = deque()

                def step_units(k):
                    while k > 0 and agen:
                        try:
                            next(agen[0])
                            k -= 1
                        except StopIteration:
                            agen.popleft()

                PACE = {0: 0, 1: 3, 2: 6, 3: 8}
                for sc in range(n_ch):
                    sl = slice(sc * CHUNK, (sc + 1) * CHUNK)
                    xs = xpool.tile([128, n_dt, CHUNK], BF16, tag="xs")
                    nq = 8 if sc == 0 else 4
                    for dq in range(nq):
                        step = n_dt // nq
                        nc.sync.dma_start(
                            out=xs[:, dq * step:(dq + 1) * step],
                            in_=x_d[:, sc, dq * step:(dq + 1) * step],
                        )
                    if sc == 0:
                        # rest of the weights behind xs chunk 0 on the fast
                        # HWDGE sync queue; all transfers fully contiguous
                        for hh in range(1, HQ):
                            nc.sync.dma_start(
                                out=wq_sb[:, hh], in_=wqT_d[:, hh]
                            )
                        nc.sync.dma_start(out=wkv_sb[:, 0], in_=wkvT_d[:, 0])
                        nc.sync.dma_start(out=wkv_sb[:, 1], in_=wkvT_d[:, 1])
                    # cols 0..3 = q heads, 4 = k, 5 = v
                    for col in range(6):
                        ps = psP.tile([128, CHUNK], F32, tag="ps",
                                      name=f"ps{sc}{col}")
                        for dt in range(n_dt):
                            if col < 4:
                                w_ap = wq_sb[:, col, dt, :]
                            elif col == 4:
                                w_ap = wkv_sb[:, 0, dt, :]
                            else:
                                w_ap = wkv_sb[:, 1, dt, :]
                            nc.tensor.matmul(
                                ps, w_ap, xs[:, dt, :],
                                start=(dt == 0), stop=(dt == n_dt - 1),
                            )
                        # copy issues right after the col group (frees the
                        # ring slot early); the swap matmul and RoPE are
                        # emitted after the interleaved attention steps so
                        # they don't block ready matmuls at the PE FIFO head.
                        raw = rope_pool.tile([128, CHUNK], BF16,
                                             tag="raw", name=f"raw{sc}{col}")
                        nc.scalar.copy(raw, ps)
                        step_units(PACE[sc])
                        if col < 5:
                            # RoPE: rot = p*t1T + swap_halves(p)*t2T; the
                            # half swap is a PE permutation matmul
                            ps_sw = psP.tile([128, CHUNK], F32, tag="ps",
                                             name=f"psw{sc}{col}")
                            nc.tensor.matmul(
                                ps_sw, pswap_sb, raw, start=True, stop=True,
                            )
                            m1 = rope_pool.tile([128, CHUNK], BF16, tag="m1")
                            nc.vector.tensor_mul(m1, raw, t1T_sb[:, sl])
                            m2 = rope_pool.tile([128, CHUNK], BF16, tag="m2")
                            nc.vector.tensor_mul(m2, ps_sw, t2T_sb[:, sl])
                            dest = (qT_sb[:, col, sl] if col < 4
                                    else kT_sb[:, sl])
                            nc.vector.tensor_add(dest, m1, m2)
                        else:
                            # v: PE-transpose [e, s] -> [s, e]
                            for j in range(spc):
                                pt = psP.tile([128, 128], BF16, tag="ps",
                                              name=f"pt{sc}{j}")
                                nc.tensor.transpose(
                                    pt, raw[:, j * 128:(j + 1) * 128], ident,
                                )
                                nc.vector.tensor_copy(
                                    v_sb[:, sc * spc + j, :], pt
                                )
                    for h in range(HQ):
                        agen.append(attn_unit_gen(sc, h))

                while agen:
                    step_units(1)
                emit_norm()
                while cq:
                    emit_c_piece()

    return nc


# ---------------------------------------------------------------------------
# Host-side prep


_ROPE_PERM = np.concatenate([np.arange(0, HEAD_DIM, 2), np.arange(1, HEAD_DIM, 2)])


def _prep_tables(freq_cis, S_=S, HD_=HEAD_DIM):
    """RoPE tables in permuted-half layout: rot = q*t1 + swap(q)*t2."""
    fc = np.asarray(freq_cis, dtype=np.float32)
    A = fc[:, :, 0, 0]
    Bm = fc[:, :, 0, 1]
    C = fc[:, :, 1, 0]
    Dm = fc[:, :, 1, 1]
    t1 = np.concatenate([A, Dm], axis=1).astype(np.float32)  # [S, HD]
    t2 = np.concatenate([Bm, C], axis=1).astype(np.float32)
    return np.ascontiguousarray(t1), np.ascontiguousarray(t2)


def _perm_head_rows(w):
    """Permute rows within each 128-row head block: evens first, odds second."""
    nh = w.shape[0] // HEAD_DIM
    return np.ascontiguousarray(
        w.reshape(nh, HEAD_DIM, -1)[:, _ROPE_PERM, :].reshape(w.shape)
    )


def _bf16(a):
    return np.ascontiguousarray(a.astype(ml_dtypes.bfloat16))


def _pmajor(a):
    """[T*128, F...] -> [128, T, F...] partition-major layout."""
    t = a.shape[0] // 128
    return np.ascontiguousarray(
        a.reshape(t, 128, *a.shape[1:]).swapaxes(0, 1)
    )


def make_core_inputs(x, freq_cis, wq, wk, wv, wo):
    """Build the 8 per-core input maps."""
    x = np.asarray(x, np.float32)
    wq = np.asarray(wq, np.float32)
    wk = np.asarray(wk, np.float32)
    wv = np.asarray(wv, np.float32)
    wo = np.asarray(wo, np.float32)
    t1, t2 = _prep_tables(freq_cis)
    t1T = _bf16(t1.T)   # [HD, S] = [e, s]
    t2T = np.ascontiguousarray(t2.T)
    # single shift-invariant causal mask: mask[j][:, off:] == mask0[:, :w]
    u = np.arange(512)[None, :]
    p = np.arange(128)[:, None]
    mask0 = np.where(u >= p, 0.0, NEG).astype(np.float32)
    ident = _bf16(np.eye(128, dtype=np.float32))
    pswap = _bf16(np.roll(np.eye(128, dtype=np.float32), 64, axis=1))
    IQ = HQ * HEAD_DIM

    in_maps = []
    for core in range(8):
        b, g = divmod(core, N_GROUPS)
        wq_g = _perm_head_rows(wq[g * IQ:(g + 1) * IQ])
        wk_g = _perm_head_rows(wk[g * HEAD_DIM:(g + 1) * HEAD_DIM])
        wv_g = wv[g * HEAD_DIM:(g + 1) * HEAD_DIM]
        wqT = np.ascontiguousarray(
            _pmajor(_bf16(wq_g.T)).reshape(128, 16, 4, 128).swapaxes(1, 2)
        )
        wkvT = np.ascontiguousarray(
            _pmajor(_bf16(np.concatenate([wk_g.T, wv_g.T], axis=1)))
            .reshape(128, 16, 2, 128).swapaxes(1, 2)
        )
        woT = _pmajor(_bf16(wo[:, g * IQ:(g + 1) * IQ].T))
        in_maps.append({
            "xT": np.ascontiguousarray(
                _pmajor(_bf16(x[b].T)).reshape(128, 16, 4, 512).swapaxes(1, 2)
            ),
            "wqT": wqT,
            "wkvT": wkvT,
            "woT": woT,
            "t1T": t1T,
            "t2T": t2T,
            "mask0": mask0,
            "ident": ident,
            "pswap": pswap,
            "ones_col": _bf16(np.ones((128, 1), np.float32)),
            "ones_row": np.ones((1, 128), np.float32),
        })
    return in_maps


_CACHED_NC = None


def _get_nc():
    global _CACHED_NC
    if _CACHED_NC is None:
        from concourse import bacc

        nc = bacc.Bacc("TRN2", target_bir_lowering=False, debug=False)
        build_attention_core(nc)
        nc.compile()
        _CACHED_NC = nc
    return _CACHED_NC


def kernel(x, freq_cis, wq, wk, wv, wo):
    from concourse.bass_utils import run_bass_kernel_spmd

    nc = _get_nc()
    in_maps = make_core_inputs(x, freq_cis, wq, wk, wv, wo)
    res = run_bass_kernel_spmd(nc, in_maps, list(range(8)))
    out = np.zeros((B, S, DIM), dtype=np.float32)
    for core in range(8):
        b = core // N_GROUPS
        out[b] += res.results[core]["out_partial"].astype(np.float32)
    return out
